# revision 48
# baseline (speedup 1.0000x reference)
"""EpisodicMemory kernel for Trainium2, 8-core data-parallel, bf16 compute.

Reference computation (per batch b, d=32, m=64 memory slots, 2 hops):
    M = vs[b]
    for hop:
        Rh[m,:] = R[b,hop,m] @ h[b,hop,m]                  # batched matvec
        z = [Rh*v, Rh*M, |Rh-v|, |Rh-M|]                   # [m, 4d]
        Z = tanh(z @ W1.T + b1) @ W2.T (+ b2: dropped — softmax-invariant)
        g = softmax(Z over m); o = sum_m ts[b,hop,m] * g[m]
        M = GRUCell(o, M)
    out[b] = M

Sharding: pure data parallel over batch; 128 batches per core.

bf16 strategy (tolerance 2e-2; measured end-to-end rel err ~1.6e-3):
  Rs/hs/ts host-converted to bf16 — halves the dominant HBM traffic
  (Rs 64 MiB -> 32 MiB per core). Einsum product in bf16; reduction via
  an in-place halving tree of TensorTensor adds (DVE runs TT at ~0.54
  ns/elem vs TensorReduce ~1.04). MLP matmuls bf16 (PSUM fp32), softmax
  stats fp32, GRU fp32.

HW-calibrated (microbenched): DVE 0.54 ns/elem for TensorTensor, Pool
1.45 ns/elem (and it serializes badly against DVE chains -> einsum is
ALL-DVE), SP DMA ring ~600 GB/s for 4 MiB transfers. The whole kernel is
software-pipelined: each group's R load + mul is emitted before the
previous group's tree (1-group lookahead), and the next hop's first two
groups are emitted before this hop's softmax/GRU section, so DVE streams
through the inter-hop serial chain. Sigmoid is computed as
0.5+0.5*tanh(x/2) so every Act function stays in the one exp_and_others
table (no LoadActFuncSet swaps). GRU gate matmuls are batched (96-wide),
softmax's sum is fused into Exp via accum_out. PSUM-accumulating matmul
pairs (start/stop split) measured catastrophically slow on HW - avoided.

Queue discipline: the SP SEQ is in-order and blocks on each DMA, so SP
carries ONLY the R stream (plus last-hop z flush + out after it). All
other traffic (h/t, packed consts, hop-0 z, gathers, M bounce) rides the
Act queue. Weights arrive packed in two tensors (one DMA each); z rows
are collected in one SBUF tile per hop and flushed with 2 DMAs.

Per-core layout: b = blk*8 + bp*4 + g; partition p = bp*64 + m.
  - R tiles [128 part=(bp,m), free=(blk4,g4,d32,e32)]: 4 blocks per DMA.
  - z_scr DRAM [blk, bp, g, m]: gather = one clean [128, 64] load.
  - v_rep/M_rep [128, (blk g d)] bf16; v_rep host-prebuilt, M_rep via
    4-DMA DRAM bounce after hop 0.
"""

import numpy as np
import ml_dtypes

import concourse.bacc as bacc
import concourse.bass as bass
import concourse.mybir as mybir
import concourse.tile as tile
from concourse.masks import make_identity

F32 = mybir.dt.float32
BF16 = mybir.dt.bfloat16
AF = mybir.ActivationFunctionType
ALU = mybir.AluOpType
AX = mybir.AxisListType

B, N_HOP, N_MEM, DIM = 1024, 2, 64, 32
N_CORES = 8
BC = B // N_CORES            # 128 batches per core
NBLK = 16                    # blocks of 8 batches
NG = 4                       # g per bp
D4 = 4 * DIM                 # 128 MLP input features
RB = 4                       # blocks per R DMA
NGRP = NBLK // RB            # 4 groups per hop
ROWS = 512                   # zt columns per block (4 g-chunks x 128)

# packed fp32 consts layout (columns)
PF_VS = 0                    # [128, 0:32]   vs rows
PF_B1 = 32                   # [0:32, 32]    b1
PF_W = 33                    # [0:32, 33+]   WihT0 WhhT0 WihT1 WhhT1 (96 each)
PF_BIAS = PF_W + 4 * 96      # [0:32, 417+]  bih0(3) bhh0(3) bih1(3) bhh1(3)
PF_HB = PF_BIAS + 12         # [0:32, 429+]  0.5*(bih+bhh) r,z per hop
PF_COLS = PF_HB + 4          # 433


def build_nc(n_iter: int = 1, stage: str = "full") -> bass.Bass:
    nc = bacc.Bacc("TRN2")

    Rs_d = nc.dram_tensor(
        "Rs", [N_HOP, NBLK, 2, N_MEM, NG, DIM, DIM], BF16, kind="ExternalInput"
    )
    hs_d = nc.dram_tensor(
        "hs", [N_HOP, 2, N_MEM, NBLK, NG, DIM], BF16, kind="ExternalInput"
    )
    ts_d = nc.dram_tensor("ts", [BC, N_HOP, DIM, N_MEM], BF16, kind="ExternalInput")
    vrep_d = nc.dram_tensor(
        "v_rep", [128, NBLK * NG * DIM], BF16, kind="ExternalInput"
    )
    pkbf_d = nc.dram_tensor("pk_bf", [128, 33], BF16, kind="ExternalInput")
    pkf_d = nc.dram_tensor("pk_f32", [128, PF_COLS], F32, kind="ExternalInput")
    out_d = nc.dram_tensor("out", [BC, DIM], F32, kind="ExternalOutput")
    m_scr = nc.dram_tensor("m_scratch", [BC, DIM], BF16)
    m_scr2 = nc.dram_tensor("m_scratch2", [2, NBLK, NG, DIM], BF16)
    m_scr3 = nc.dram_tensor("m_scratch3", [128, NBLK * NG * DIM], BF16)
    z_scr = nc.dram_tensor("z_scratch", [NBLK, 2, NG, N_MEM], BF16)

    import contextlib

    with tile.TileContext(nc) as tc:
        with (
            (tc.For_i(0, n_iter, 1) if n_iter > 1 else contextlib.nullcontext()),
            tc.tile_pool(name="consts", bufs=1) as consts,
            tc.tile_pool(name="hop_io", bufs=2) as hop_io,
            tc.tile_pool(name="rpool", bufs=4) as rpool,
            tc.tile_pool(name="fpool", bufs=3) as fpool,
            tc.tile_pool(name="zpool", bufs=3) as zpool,
            tc.tile_pool(name="apool", bufs=3) as apool,
            tc.tile_pool(name="small", bufs=2) as small,
            tc.tile_pool(name="mstate", bufs=2) as mstate,
            tc.tile_pool(name="pp_z", bufs=2, space="PSUM") as pp_z,
            tc.tile_pool(name="pp_1", bufs=2, space="PSUM") as pp_1,
            tc.tile_pool(name="pp_2", bufs=2, space="PSUM") as pp_2,
            tc.tile_pool(name="pp_g", bufs=2, space="PSUM") as pp_g,
        ):
            ident = consts.tile([128, 128], F32)
            make_identity(nc, ident)
            ident_bf = consts.tile([128, 128], BF16)
            make_identity(nc, ident_bf)

            # preload h/t for both hops (Act queue) so the hop-1 einsum never
            # stalls on them behind hop-0's z writes
            h_hops, t_hops = [], []
            for hop in range(N_HOP):
                h_hop = hop_io.tile([128, NBLK * NG * DIM], BF16, tag="h_hop")
                nc.scalar.dma_start(
                    out=h_hop,
                    in_=hs_d[hop].rearrange("bp m blk g e -> (bp m) (blk g e)"),
                )
                h_hops.append(h_hop)
                # t d-major [b, (d m)] so the m-reduce is innermost-packed
                t_hop = hop_io.tile([BC, DIM * N_MEM], BF16, tag="t_hop")
                nc.scalar.dma_start(
                    out=t_hop, in_=ts_d[:, hop].rearrange("b d m -> b (d m)")
                )
                t_hops.append(t_hop)

            v_rep = consts.tile([128, NBLK * NG * DIM], BF16)
            nc.scalar.dma_start(out=v_rep, in_=vrep_d[:, :])
            M_rep = v_rep  # hop 0: M == vs

            # ---- packed weights: 2 DMAs total ----
            pk_bf = consts.tile([128, 33], BF16)
            nc.scalar.dma_start(out=pk_bf, in_=pkbf_d[:, :])
            pk_f = consts.tile([128, PF_COLS], F32)
            nc.scalar.dma_start(out=pk_f, in_=pkf_d[:, :])

            W1T = pk_bf[:, 0:32]
            W2T = pk_bf[0:DIM, 32:33]
            b1T = pk_f[0:DIM, PF_B1 : PF_B1 + 1]
            WihT = [pk_f[0:DIM, PF_W + 192 * h : PF_W + 192 * h + 96]
                    for h in range(N_HOP)]
            WhhT = [pk_f[0:DIM, PF_W + 192 * h + 96 : PF_W + 192 * h + 192]
                    for h in range(N_HOP)]

            hb_rz, bihn_t, bhhn_t = [], [], []
            for hop in range(N_HOP):
                c0 = PF_BIAS + 6 * hop
                gate_b = [pk_f[0:DIM, c0 + j : c0 + j + 1] for j in range(6)]
                hb = PF_HB + 2 * hop
                hb_rz.append(
                    (pk_f[0:DIM, hb : hb + 1], pk_f[0:DIM, hb + 1 : hb + 2])
                )
                bihn_t.append(gate_b[2])
                bhhn_t.append(gate_b[5])

            # ---- initial M state ----
            vs_row = pk_f[:, PF_VS : PF_VS + 32]
            vst_ps = pp_g.tile([DIM, BC], F32, tag="gpsum")
            nc.tensor.transpose(vst_ps, vs_row, ident)
            vsT = consts.tile([DIM, BC], F32)
            nc.scalar.copy(out=vsT, in_=vst_ps)
            MT = vsT  # current M^T [d, b]

            z_alls = []
            for h_ in range(N_HOP):
                # block blk lives at partition (blk%4)*32 (legal start
                # partitions), col chunk blk//4 — 4 KB/partition instead of
                # 16 KB for a [1, 8192] single-partition tile
                z_all_t = zpool.tile(
                    [128, 4 * ROWS], BF16, tag=f"z_all{h_}", bufs=1
                )
                z_alls.append(z_all_t)
            M_reps = {0: v_rep}
            MT_box = [vsT]  # current M^T [d, b]

            def group_head(hop, grp):
                blk0 = grp * RB
                r_tile = rpool.tile([128, RB * NG * DIM * DIM], BF16, tag="R")
                # SP carries ONLY the R stream: its in-order SEQ must never
                # wait on downstream compute
                nc.sync.dma_start(
                    out=r_tile.rearrange("p (blk f) -> p blk f", blk=RB),
                    in_=Rs_d[hop, blk0 : blk0 + RB].rearrange(
                        "blk bp m g d e -> (bp m) blk (g d e)"
                    ),
                )
                # P = R * h (in-place, DVE), h broadcast over d
                r5 = r_tile.rearrange(
                    "p (blk g d e) -> p blk g d e", blk=RB, g=NG, d=DIM
                )
                h_v = (
                    h_hops[hop][:, blk0 * NG * DIM : (blk0 + RB) * NG * DIM]
                    .rearrange("p (blk g e) -> p blk g e", blk=RB, g=NG)
                    .unsqueeze(3)
                    .broadcast_to((128, RB, NG, DIM, DIM))
                )
                nc.vector.tensor_mul(r5, r5, h_v)
                return hop, blk0, r_tile

            def group_tail(hop, blk0, r_tile):
                # Rh[(bp,m), (blk,g,d)] = sum_e P via in-place halving tree
                # (TT adds run 2 elem/cycle on DVE vs TensorReduce's 1)
                rh = fpool.tile([128, RB * NG * DIM], BF16, tag="rh", bufs=2)
                v3 = r_tile.rearrange("p (gd e) -> p gd e", e=DIM)
                for w in (16, 8, 4, 2):
                    nc.vector.tensor_add(
                        v3[:, :, 0:w], v3[:, :, 0:w], v3[:, :, w : 2 * w]
                    )
                nc.vector.tensor_add(rh, v3[:, :, 0], v3[:, :, 1])
                # features F [(bp,m), (blk, g, f, d)]
                f_grp = fpool.tile([128, RB * NG * 4 * DIM], BF16, tag="F", bufs=2)
                f5 = f_grp.rearrange(
                    "p (blk g f d) -> p blk g f d", blk=RB, g=NG, f=4
                )
                rh4 = rh.rearrange("p (blk g d) -> p blk g d", blk=RB, g=NG)
                vr4 = v_rep[
                    :, blk0 * NG * DIM : (blk0 + RB) * NG * DIM
                ].rearrange("p (blk g d) -> p blk g d", blk=RB, g=NG)
                mr4 = M_reps[hop][
                    :, blk0 * NG * DIM : (blk0 + RB) * NG * DIM
                ].rearrange("p (blk g d) -> p blk g d", blk=RB, g=NG)
                nc.vector.tensor_mul(f5[:, :, :, 0, :], rh4, vr4)
                nc.vector.tensor_mul(f5[:, :, :, 1, :], rh4, mr4)
                nc.vector.tensor_sub(f5[:, :, :, 2, :], rh4, vr4)
                nc.vector.tensor_sub(f5[:, :, :, 3, :], rh4, mr4)
                nc.scalar.activation(f5[:, :, :, 2, :], f5[:, :, :, 2, :], AF.Abs)
                nc.scalar.activation(f5[:, :, :, 3, :], f5[:, :, :, 3, :], AF.Abs)
                if stage != "einsum":
                    mlp_tail(hop, blk0, f_grp)

            def mlp_tail(hop, blk0, f_grp):
                z_all = z_alls[hop]
                for pair in range(RB // 2):
                    # transpose 2 blocks into one PSUM bank ([128, 1024] bf16
                    # = 2 KB/partition) -> single Act copy-out
                    zt_ps = pp_z.tile([D4, 2 * ROWS], BF16, tag="zt")
                    for j in range(2):
                        blk = pair * 2 + j
                        for g in range(NG):
                            c0 = (blk * NG + g) * 128
                            nc.tensor.transpose(
                                zt_ps[
                                    :, j * ROWS + g * 128 : j * ROWS + (g + 1) * 128
                                ],
                                f_grp[:, c0 : c0 + 128],
                                ident_bf,
                            )
                    zt_sb = zpool.tile([D4, 2 * ROWS], BF16, tag="zt_sb", bufs=2)
                    nc.scalar.copy(out=zt_sb, in_=zt_ps)

                    for j in range(2):
                        blk = blk0 + pair * 2 + j
                        ps1 = pp_1.tile([DIM, ROWS], F32, tag="ps1")
                        nc.tensor.matmul(
                            ps1,
                            lhsT=W1T,
                            rhs=zt_sb[:, j * ROWS : (j + 1) * ROWS],
                            start=True,
                            stop=True,
                        )
                        a1 = apool.tile([DIM, ROWS], BF16, tag="a1", bufs=2)
                        nc.scalar.activation(a1, ps1, AF.Tanh, bias=b1T)
                        ps2 = pp_2.tile([1, ROWS], F32, tag="ps2")
                        nc.tensor.matmul(
                            ps2, lhsT=W2T, rhs=a1, start=True, stop=True
                        )
                        # collect z at partition (blk%4)*32, col chunk blk//4
                        q, ch = (blk % 4) * 32, blk // 4
                        nc.scalar.copy(
                            out=z_all[q : q + 1, ch * ROWS : (ch + 1) * ROWS],
                            in_=ps2,
                        )
                        # flush immediately on the Act ring (idle during the
                        # einsum) so the hop-end gather waits only on the
                        # final block, not 16 queued DMAs
                        nc.scalar.dma_start(
                            out=z_scr[blk]
                            .unsqueeze(0)
                            .rearrange("o bp g m -> o g bp m"),
                            in_=z_all[
                                q : q + 1, ch * ROWS : (ch + 1) * ROWS
                            ].rearrange("o (g bp m) -> o g bp m", g=NG, bp=2),
                        )

            def hop_tail(hop):
                if stage == "einsum":
                    if hop == N_HOP - 1:
                        dummy = small.tile([BC, DIM], F32, tag="dummy")
                        nc.scalar.copy(out=dummy, in_=h_hops[hop][:, 0:DIM])
                        nc.sync.dma_start(out=out_d[:, :], in_=dummy)
                    return
                # flush z: z_all rows (blk) x free (g, bp, m) -> z_scr
                # [blk, bp, g, m] in one DMA (partition dim = blk)
                zq = nc.sync if hop == N_HOP - 1 else nc.scalar
                # gather Z rows: z_scr partition (blk bp g) == natural b
                Z_row = small.tile([BC, N_MEM], BF16, tag="Z_row")
                zq.dma_start(
                    out=Z_row,
                    in_=z_scr.rearrange("blk bp g m -> (blk bp g) m"),
                )
                if stage == "mlp":
                    if hop == N_HOP - 1:
                        dummy = small.tile([BC, DIM], F32, tag="dummy")
                        nc.scalar.copy(out=dummy, in_=Z_row[:, 0:DIM])
                        nc.sync.dma_start(out=out_d[:, :], in_=dummy)
                    return

                # softmax over m, batched over all 128 b; sum fused into the
                # Exp via accum_out
                nmx = small.tile([BC, 1], F32, tag="nmx")
                nc.vector.tensor_reduce(
                    out=nmx, in_=Z_row, axis=AX.X, op=ALU.max, negate=True
                )
                e_row = small.tile([BC, N_MEM], BF16, tag="e_row")
                ssum = small.tile([BC, 1], F32, tag="ssum")
                nc.scalar.activation(
                    e_row, Z_row, AF.Exp, bias=nmx, accum_out=ssum
                )
                rsum = small.tile([BC, 1], F32, tag="rsum")
                nc.vector.reciprocal(rsum, ssum)

                # o[b,d] = (sum_m t[b,d,m] * e[b,m]) / ssum[b]; the divide is
                # applied to the tiny o_row so the reciprocal runs off-chain
                t3 = t_hops[hop].rearrange("b (d m) -> b d m", d=DIM)
                e3 = e_row.unsqueeze(1).broadcast_to((BC, DIM, N_MEM))
                nc.vector.tensor_mul(t3, t3, e3)
                o_raw = small.tile([BC, DIM], F32, tag="o_raw")
                nc.vector.tensor_reduce(out=o_raw, in_=t3, axis=AX.X, op=ALU.add)
                o_row = small.tile([BC, DIM], F32, tag="o_row")
                nc.vector.tensor_scalar_mul(o_row, o_raw, rsum)

                # GRU (transposed layout [*, b], fp32); all 3 gates' input /
                # hidden matmuls batched (96-wide lhsT)
                MT = MT_box[0]
                ot_ps = pp_g.tile([DIM, BC], F32, tag="gpsum")
                nc.tensor.transpose(ot_ps, o_row, ident)
                oT = small.tile([DIM, BC], F32, tag="oT")
                nc.scalar.copy(out=oT, in_=ot_ps)

                gi_all = pp_g.tile([3 * DIM, BC], F32, tag="gpsum")
                nc.tensor.matmul(
                    gi_all, lhsT=WihT[hop], rhs=oT, start=True, stop=True
                )
                gh_all = pp_g.tile([3 * DIM, BC], F32, tag="gpsum")
                nc.tensor.matmul(
                    gh_all, lhsT=WhhT[hop], rhs=MT, start=True, stop=True
                )

                rz_t = []
                for g in range(2):
                    gi = gi_all[g * DIM : (g + 1) * DIM, :]
                    gh = gh_all[g * DIM : (g + 1) * DIM, :]
                    gb = small.tile([DIM, BC], F32, tag=f"g{g}b")
                    nc.vector.tensor_scalar(
                        out=gb, in0=gi, scalar1=0.0, scalar2=None, op0=ALU.add
                    )
                    nc.vector.tensor_add(gb, gb, gh)
                    gt = small.tile([DIM, BC], F32, tag=f"gate{g}")
                    # sigmoid via tanh keeps every Act func in the
                    # exp_and_others table (no table swaps); hb is the
                    # pre-halved bias: tanh(x/2 + b/2)
                    nc.scalar.activation(
                        gt, gb, AF.Tanh, scale=0.5, bias=hb_rz[hop][g]
                    )
                    nc.vector.tensor_scalar(
                        out=gt, in0=gt, scalar1=0.5, scalar2=0.5,
                        op0=ALU.mult, op1=ALU.add,
                    )
                    rz_t.append(gt)
                r_t, z_t = rz_t

                gi_n = gi_all[2 * DIM : 3 * DIM, :]
                gh_n = gh_all[2 * DIM : 3 * DIM, :]
                ghn = small.tile([DIM, BC], F32, tag="ghn")
                nc.vector.tensor_scalar(
                    out=ghn, in0=gh_n, scalar1=bhhn_t[hop], scalar2=None,
                    op0=ALU.add,
                )
                gin = small.tile([DIM, BC], F32, tag="gin")
                nc.vector.tensor_scalar(
                    out=gin, in0=gi_n, scalar1=bihn_t[hop], scalar2=None,
                    op0=ALU.add,
                )
                n1 = small.tile([DIM, BC], F32, tag="n1")
                nc.vector.tensor_mul(n1, r_t, ghn)
                nc.vector.tensor_add(n1, n1, gin)
                n_t = small.tile([DIM, BC], F32, tag="n_t")
                nc.scalar.activation(n_t, n1, AF.Tanh)

                # M' = n + z * (M - n)
                MT_new = mstate.tile([DIM, BC], F32, tag="MT")
                nc.vector.tensor_sub(MT_new, MT, n_t)
                nc.vector.tensor_mul(MT_new, MT_new, z_t)
                nc.vector.tensor_add(MT_new, MT_new, n_t)
                MT_box[0] = MT_new

                # M_row for output / M_rep rebuild
                mrow_ps = pp_g.tile([BC, DIM], F32, tag="gpsum")
                nc.tensor.transpose(mrow_ps, MT_new, ident[:DIM, :DIM])
                M_row = mstate.tile([BC, DIM], F32, tag="M_row")
                nc.scalar.copy(out=M_row, in_=mrow_ps)

                if hop < N_HOP - 1:
                    # rebuild M_rep (bf16) via DRAM bounce
                    Mb = mstate.tile([BC, DIM], BF16, tag="Mb")
                    nc.scalar.copy(out=Mb, in_=M_row)
                    nc.scalar.dma_start(out=m_scr[:, :], in_=Mb)
                    # DRAM->DRAM permute: natural b -> [bp, blk, g, d]
                    nc.scalar.dma_start(
                        out=m_scr2[:, :, :, :],
                        in_=m_scr.rearrange(
                            "(blk bp g) d -> bp blk g d", bp=2, g=NG
                        ),
                    )
                    # partition_broadcast prepends the broadcast dim: in is
                    # [m(bcast), bp, f]; write rows r = bp*64+m accordingly
                    nc.scalar.dma_start(
                        out=m_scr3.rearrange("(bp m) f -> m bp f", bp=2),
                        in_=m_scr2.rearrange(
                            "bp blk g d -> bp (blk g d)"
                        ).partition_broadcast(N_MEM),
                    )
                    M_rep_new = mstate.tile(
                        [128, NBLK * NG * DIM], BF16, tag="M_rep", bufs=1
                    )
                    nc.scalar.dma_start(out=M_rep_new, in_=m_scr3[:, :])
                    M_reps[hop + 1] = M_rep_new
                else:
                    nc.sync.dma_start(out=out_d[:, :], in_=M_row)

            # cross-hop software-pipelined driver: the next group's R load +
            # mul are emitted before the previous group's tree (keeps DVE's
            # in-order stream busy), and the next HOP's first LA heads are
            # emitted before this hop's softmax/GRU section so the einsum
            # keeps streaming through the inter-hop serial chain.
            LA = 3
            pend = []
            for hop in range(N_HOP):
                start = 0 if hop == 0 else LA
                for grp in range(start, NGRP):
                    pend.append(group_head(hop, grp))
                    if len(pend) >= 2:
                        group_tail(*pend.pop(0))
                if hop < N_HOP - 1:
                    for g2 in range(LA):
                        pend.append(group_head(hop + 1, g2))
                        if pend[0][0] == hop:
                            group_tail(*pend.pop(0))
                    hop_tail(hop)
                else:
                    while pend:
                        group_tail(*pend.pop(0))
                    hop_tail(hop)

    nc.compile()
    return nc


_NC_CACHE = None


def _get_nc():
    global _NC_CACHE
    if _NC_CACHE is None:
        _NC_CACHE = build_nc()
    return _NC_CACHE


BF_NP = ml_dtypes.bfloat16


def make_in_maps(hs, Rs, ts, vs, W1, b1, W2, W_ih, W_hh, b_ih, b_hh):
    hs = np.asarray(hs)
    Rs = np.asarray(Rs)
    ts = np.asarray(ts)
    vs = np.asarray(vs)
    W1 = np.asarray(W1)
    b1 = np.asarray(b1)
    W2 = np.asarray(W2)
    W_ih = np.asarray(W_ih)
    W_hh = np.asarray(W_hh)
    b_ih = np.asarray(b_ih)
    b_hh = np.asarray(b_hh)

    # packed bf16 consts [128, 33]: W1T | W2T column
    pk_bf = np.zeros((128, 33), dtype=np.float32)
    pk_bf[:, 0:32] = W1.T
    pk_bf[0:DIM, 32] = W2[0, :]
    pk_bf = pk_bf.astype(BF_NP)

    # packed fp32 consts [128, PF_COLS] (vs slot filled per core below)
    pk_f = np.zeros((128, PF_COLS), dtype=np.float32)
    pk_f[0:DIM, PF_B1] = b1
    for h in range(N_HOP):
        pk_f[0:DIM, PF_W + 192 * h : PF_W + 192 * h + 96] = W_ih[h].T
        pk_f[0:DIM, PF_W + 192 * h + 96 : PF_W + 192 * h + 192] = W_hh[h].T
        for j in range(3):
            pk_f[0:DIM, PF_BIAS + 6 * h + j] = b_ih[h, DIM * j : DIM * (j + 1)]
            pk_f[0:DIM, PF_BIAS + 6 * h + 3 + j] = b_hh[h, DIM * j : DIM * (j + 1)]
        for g in range(2):
            pk_f[0:DIM, PF_HB + 2 * h + g] = 0.5 * (
                b_ih[h, DIM * g : DIM * (g + 1)] + b_hh[h, DIM * g : DIM * (g + 1)]
            )

    in_maps = []
    for c in range(N_CORES):
        sl = slice(c * BC, (c + 1) * BC)
        # Rs [BC, hop, m, d, e] -> [hop, blk, bp, m, g, d, e]
        Rp = (
            Rs[sl]
            .reshape(NBLK, 2, NG, N_HOP, N_MEM, DIM, DIM)
            .transpose(3, 0, 1, 4, 2, 5, 6)
            .astype(BF_NP)
        )
        # hs [BC, hop, m, e] -> [hop, bp, m, blk, g, e]
        hp = (
            hs[sl]
            .reshape(NBLK, 2, NG, N_HOP, N_MEM, DIM)
            .transpose(3, 1, 4, 0, 2, 5)
            .astype(BF_NP)
        )
        # ts [BC, hop, m, d] -> [BC, hop, d, m]
        tp = ts[sl].transpose(0, 1, 3, 2).astype(BF_NP)
        vsc = np.ascontiguousarray(vs[sl])
        # v_rep [128=(bp,m), (blk, g, d)]
        vv = vsc.reshape(NBLK, 2, NG, DIM).transpose(1, 0, 2, 3)  # [bp, blk, g, d]
        v_rep = (
            np.broadcast_to(vv[:, None], (2, N_MEM, NBLK, NG, DIM))
            .reshape(128, NBLK * NG * DIM)
            .astype(BF_NP)
        )
        pk_fc = pk_f.copy()
        pk_fc[:, PF_VS : PF_VS + 32] = vsc
        in_maps.append(
            {
                "Rs": np.ascontiguousarray(Rp),
                "hs": np.ascontiguousarray(hp),
                "ts": np.ascontiguousarray(tp),
                "v_rep": np.ascontiguousarray(v_rep),
                "pk_bf": pk_bf,
                "pk_f32": pk_fc,
            }
        )
    return in_maps


def kernel(hs, Rs, ts, vs, W1, b1, W2, b2, W_ih, W_hh, b_ih, b_hh):
    from concourse.bass_utils import run_bass_kernel_spmd

    nc = _get_nc()
    in_maps = make_in_maps(hs, Rs, ts, vs, W1, b1, W2, W_ih, W_hh, b_ih, b_hh)
    res = run_bass_kernel_spmd(nc, in_maps, list(range(N_CORES)))
    return np.concatenate([r["out"] for r in res.results], axis=0)
